# revision 20
# baseline (speedup 1.0000x reference)
"""GATNet (4-layer GAT + pooling head) on 8 Trainium2 NeuronCores.

Strategy (per sharding hint): partition nodes across the 8 cores; each core
owns a contiguous block of 2500 (renumbered) nodes and all edges whose dst
lands in that block.  Per layer each core computes its own node table
(h@W plus attention-logit exponentials); the tables are AllGathered into a
full HBM gather table (in chunks, overlapped with the previous layer's edge
phase), and each core gathers the source-node rows for its incoming edges
with dma_gather (dst-lane layout: edge slot s of dst-lane p lands at
partition p, free block s).  Edge softmax is restructured so no per-edge
dst-indexed data is ever needed:

    ref:  e = leaky_relu(asrc[src]+adst[dst]); alpha = softmax_dst(e)
    here: m = max(exp(asrc[src]), exp(0.2*asrc[src]) * exp(-0.8*adst[dst]))
          == exp(leaky_relu(asrc+adst)) * exp(-adst)   (dst factor cancels
          in the softmax), with exp(asrc)/exp(0.2*asrc) carried inside the
          gathered row and exp(-0.8*adst) a per-own-node column broadcast.

Segment sums become per-partition free-dim reductions.  The pooling head
(function-node sum pool + decision-var select) is folded into per-tile
one-hot matmuls accumulated in PSUM during the layer-4 loop, followed by a
single tiny AllReduce.  All graph index tables are built on the host
(compile-time) from the runtime edge_index.
"""

import os
import sys

import numpy as np

for _p in ("/opt/trn_rl_repo", "/root/.axon_site/_ro/trn_rl_repo"):
    if os.path.isdir(_p) and _p not in sys.path:
        sys.path.insert(0, _p)

import concourse.bacc as bacc
import concourse.bass as bass
import concourse.tile as tile
from concourse import mybir
from concourse.bass_utils import run_bass_kernel_spmd

F32 = mybir.dt.float32
BF16 = mybir.dt.float16  # fp16: 11-bit mantissa, plenty of range here
I16 = mybir.dt.int16
AF = mybir.ActivationFunctionType
ALU = mybir.AluOpType

N_NODES = 20000
N_EDGES = 320000
NCORES = 8
NOWN = N_NODES // NCORES          # 2500
NTILE = (NOWN + 127) // 128       # 20 dst tiles per core
NPAD = NTILE * 128                # 2560 padded own nodes
SLABR = NPAD + 16                 # per-core slab rows (last 16 all-zero)
DUMMY_ROW = NPAD                  # core 0's first zero row
TAB_ROWS = NCORES * SLABR         # 20608

# layer configs: (heads, ch, cin, cout)
LCFG = [(8, 8, 16, 64), (8, 8, 64, 64), (8, 8, 64, 64), (4, 64, 64, 256)]

# edge-phase slot-chunk sizes (free-dim S per gather): keep SBUF bounded
S_CHUNK = [32, 32, 32, 24]

# packed f32 constant-block column offsets
OFF_WEXT = [0, 80, 160]           # [cin, 80] each
OFF_WSD4 = 240                    # [64, 8]
OFF_WST = 248                     # [128, 2*64]
OFF_B = 376                       # 4 x [128, 64]
OFF_WP = 632                      # [64, 64]
OFF_WT = 696                      # [64, 64]
OFF_WO = 760                      # [128, 1]
OFF_BO = 761                      # [128, 1]
OFF_PADM = 762                    # [128, 1]
OFF_IDENT = 763                   # [128, 128]
OFF_FDV = 891                     # [128, NTILE*16]
CW = OFF_FDV + NTILE * 16         # 1211


def _idx_layout(flat):
    """Gather idx order i -> SBUF int16 [128, len/16] (pos [i%16, i//16],
    replicated over the 8 q7 core groups)."""
    flat = np.asarray(flat, np.int16)
    assert len(flat) % 16 == 0
    a = flat.reshape(-1, 16).T  # [16, n/16]
    return np.tile(a, (8, 1)).copy()


def _block_diag(a):
    """a [heads, ch] -> [heads*ch, heads] with column h = a[h] on its block."""
    heads, ch = a.shape
    out = np.zeros((heads * ch, heads), np.float32)
    for h in range(heads):
        out[h * ch:(h + 1) * ch, h] = a[h]
    return out


def _rowid(g):
    """tab row of renumbered-global node id (t-major rows within core)."""
    c, l = g // NOWN, g % NOWN
    return c * SLABR + l  # l == t*128 + p, t-major



def _preprocess(edge_index, function_idx, flag, decision_var_idxes):
    # NOTE: the appended self-loops are NOT put in the edge tables —
    # the device adds the self term analytically from local node data.
    src = np.asarray(edge_index[0], np.int64)
    dst = np.asarray(edge_index[1], np.int64)

    deg = np.bincount(dst, minlength=N_NODES)

    # renumber: global degree-desc order, round-robin over cores, so every
    # core's tile t covers the same global degree band (minimises slot pad)
    ranks = np.argsort(-deg, kind="stable")
    new_of_orig = np.empty(N_NODES, np.int64)
    gi = np.arange(N_NODES)
    new_of_orig[ranks] = (gi % NCORES) * NOWN + gi // NCORES

    src_n = new_of_orig[src]
    dst_n = new_of_orig[dst]

    deg_n = np.zeros(N_NODES, np.int64)
    np.add.at(deg_n, dst_n, 1)

    # shared tile slot counts S_t = max over cores of max degree in tile
    degs_2d = deg_n.reshape(NCORES, NOWN)
    S = []
    for t in range(NTILE):
        hi = min((t + 1) * 128, NOWN)
        S.append(int(degs_2d[:, t * 128:hi].max()))
    S = [max(s, 1) for s in S]

    # per-core edge slot tables A[l, s] = rowid(src) (DUMMY_ROW pad)
    order = np.argsort(dst_n, kind="stable")
    dst_s, src_s = dst_n[order], src_n[order]
    starts = np.searchsorted(dst_s, np.arange(N_NODES))
    slot = np.arange(len(dst_s)) - starts[dst_s]
    src_row = _rowid(src_s)

    idx_tabs = []
    for c in range(NCORES):
        m = (dst_s >= c * NOWN) & (dst_s < (c + 1) * NOWN)
        l = dst_s[m] - c * NOWN
        A = np.full((NPAD, max(S)), DUMMY_ROW, np.int64)
        A[l, slot[m]] = src_row[m]
        parts = []
        for t in range(NTILE):
            blk = A[t * 128:(t + 1) * 128, :S[t]].T  # [S_t, 128]
            parts.append(blk.reshape(-1))
        idx_tabs.append(_idx_layout(np.concatenate(parts)))

    # head one-hots: fdv[c, p, t, 0:8] = function-pool multiplicity per
    # graph; fdv[c, p, t, 8+j] = 1 if node is decision var of graph j
    fdv = np.zeros((NCORES, 128, NTILE, 16), np.float32)
    fidx_new = new_of_orig[np.asarray(function_idx, np.int64)]
    fc, fl = fidx_new // NOWN, fidx_new % NOWN
    ft, fp = fl // 128, fl % 128
    np.add.at(fdv, (fc, fp, ft, np.asarray(flag, np.int64)), 1.0)
    dvn = new_of_orig[np.asarray(decision_var_idxes, np.int64)]
    dc, dl = dvn // NOWN, dvn % NOWN
    dt, dp = dl // 128, dl % 128
    for j in range(NCORES):
        fdv[dc[j], dp[j], dt[j], 8 + j] += 1.0

    return dict(new_of_orig=new_of_orig, S=S, idx_tabs=idx_tabs, fdv=fdv)


def _build_kernel(S, idx_width):
    """Build the SPMD bass program (same for all cores).

    Per-layer gather-table rows are uniformly 128 fp16 (=256B):
    [feat(64) | asrc(h) | adst(h) | 0-pad], where feat is hW for layers
    1-3 and raw h3 for layer 4 (W4 is applied after aggregation via
    linearity: sum_e m_e*(h3@W4) == (sum_e m_e*h3)@W4).
    """
    nc = bacc.Bacc("TRN2", target_bir_lowering=False, debug=False,
                   num_devices=NCORES, num_swdge_queues=4)

    # ---- external inputs ----
    xT = nc.dram_tensor("xT", [16, NPAD], F32, kind="ExternalInput")
    idx_edges = nc.dram_tensor("idx_edges", [128, idx_width], I16,
                               kind="ExternalInput")
    cpack_d = nc.dram_tensor("cpack", [128, CW], F32, kind="ExternalInput")

    out_final = nc.dram_tensor("out_final", [8, 1], F32, kind="ExternalOutput")

    with tile.TileContext(nc) as tc:
        torder = list(range(NTILE))
        gq = [0]  # rotating SWDGE queue: each queue uses its own Q7 core
        # pair, so gathers on different queues generate descriptors in
        # parallel (dma_gather ucode: cpu_id/2 == queue_num does the work)
        with (
            tc.tile_pool(name="dram", bufs=1, space="DRAM") as dram,
            tc.tile_pool(name="const", bufs=1) as cpool,
            tc.tile_pool(name="state", bufs=1) as spool,
            tc.tile_pool(name="gather", bufs=6) as gpool,
            tc.tile_pool(name="msg", bufs=3) as mpool,
            tc.tile_pool(name="small", bufs=6) as tpool,
            tc.tile_pool(name="psum", bufs=2, space="PSUM") as ppool,
            tc.tile_pool(name="psacc", bufs=1, space="PSUM") as papool,
            tc.tile_pool(name="psumT", bufs=2, space="PSUM") as ptpool,
            tc.tile_pool(name="pshead", bufs=1, space="PSUM") as phpool,
        ):
            # ---- DRAM internals ----
            slab123 = dram.tile([SLABR, 128], BF16, tag="slab123")
            slab4 = dram.tile([SLABR, 128], BF16, tag="slab4")
            tabs = []
            for _l in range(4):
                tab_l = dram.tile([TAB_ROWS, 128], BF16, tag=f"tab{_l}",
                                  addr_space="Shared", name=f"tab{_l}")
                tabs.append(tab_l)

            # ---- load constants (one packed DMA + the idx table) ----
            cpk = cpool.tile([128, CW], F32, tag="cpk")
            nc.sync.dma_start(cpk[:], cpack_d[:, :])
            idxe_sb = cpool.tile([128, idx_width], I16, tag="idxe")
            nc.sync.dma_start(idxe_sb[:], idx_edges[:, :])

            w_sb = [cpk[0:(64 if l else 16), OFF_WEXT[l]:OFF_WEXT[l] + 80]
                    for l in range(3)]
            wsd4_sb = cpk[0:64, OFF_WSD4:OFF_WSD4 + 8]
            b_sb = [cpk[:, OFF_B + 64 * l:OFF_B + 64 * (l + 1)]
                    for l in range(4)]
            wp_sb = cpk[0:64, OFF_WP:OFF_WP + 64]
            wt_sb = cpk[0:64, OFF_WT:OFF_WT + 64]
            wo_sb = cpk[:, OFF_WO:OFF_WO + 1]
            bo8 = cpk[0:8, OFF_BO:OFF_BO + 1]
            padmask_sb = cpk[:, OFF_PADM:OFF_PADM + 1]
            ident_sb = cpk[:, OFF_IDENT:OFF_IDENT + 128]
            ident16 = cpk[0:16, OFF_IDENT:OFF_IDENT + 16]
            ident8 = cpk[0:8, OFF_IDENT:OFF_IDENT + 8]

            def wst_sb(j):
                return cpk[:, OFF_WST + 64 * j:OFF_WST + 64 * (j + 1)]

            def fdv_sb(t):
                return cpk[:, OFF_FDV + 16 * t:OFF_FDV + 16 * (t + 1)]

            zero_sb = cpool.tile([128, 128], BF16, tag="zero")
            nc.vector.memset(zero_sb[:], 0.0)
            negone = cpool.tile([128, 1], F32, tag="negone")
            nc.vector.memset(negone[:], -1.0)
            # zero the trailing pad rows of each slab (gather dummy target)
            nc.sync.dma_start(slab123[NPAD:SLABR, :], zero_sb[0:16, :])
            nc.sync.dma_start(slab4[NPAD:SLABR, :], zero_sb[0:16, :])

            # ---- persistent state ----
            zT = spool.tile([64, NPAD], F32, tag="zT")
            nc.vector.memset(zT[:], 0.0)
            nc.sync.dma_start(zT[0:16, :], xT[:, :])

            ntab_sb = spool.tile([128, NTILE, 128], BF16, tag="ntab")
            nc.vector.memset(ntab_sb[:], 0.0)
            msel = spool.tile([128, NTILE, 8], F32, tag="msel")
            zbuf = spool.tile([128, NTILE, 256], F32, tag="zbuf")
            zm = spool.tile([128, NTILE, 64], F32, tag="zm")
            cvals = spool.tile([128, NTILE, 8], BF16, tag="cvals")
            denom = spool.tile([128, NTILE, 8], F32, tag="denom")
            drec = spool.tile([128, NTILE, 8], F32, tag="drec")

            col0 = np.cumsum([0] + [8 * s for s in S]).tolist()

            def node_tile(l, t):
                """ntab rows [feat|asrc|adst] + exps + slab write, tile t."""
                h = LCFG[l][0]
                if l < 3:
                    cin = 64 if l else 16
                    ps = ppool.tile([128, 80], F32, tag="pnode")
                    nc.tensor.matmul(ps[:], zT[0:cin, t * 128:(t + 1) * 128],
                                     w_sb[l], start=True, stop=True)
                    nc.scalar.copy(ntab_sb[:, t, 0:80], ps[:])
                else:
                    nc.scalar.copy(ntab_sb[:, t, 0:64], zm[:, t, :])
                    ps = ppool.tile([128, 8], F32, tag="pnode")
                    nc.tensor.matmul(ps[:], zT[0:64, t * 128:(t + 1) * 128],
                                     wsd4_sb, start=True, stop=True)
                    nc.scalar.copy(ntab_sb[:, t, 64:72], ps[:])
                asrc = ntab_sb[:, t, 64:64 + h]
                adst = ntab_sb[:, t, 64 + h:64 + 2 * h]
                nc.scalar.activation(cvals[:, t, 0:h], adst, AF.Exp,
                                     scale=-0.8)
                nc.scalar.activation(adst, asrc, AF.Exp, scale=0.2)
                nc.scalar.activation(asrc, asrc, AF.Exp)
                slab = slab123 if l < 3 else slab4
                nc.sync.dma_start(slab[t * 128:(t + 1) * 128, :],
                                  ntab_sb[:, t, :])

            def finish_tables(l):
                slab = slab123 if l < 3 else slab4
                nc.gpsimd.collective_compute(
                    "AllGather", ALU.bypass,
                    replica_groups=[list(range(NCORES))],
                    ins=[slab[:, :].opt()],
                    outs=[tabs[l][:, :].opt()],
                )

            # layer-0 node phase: per-tile matmuls, then batched exps and a
            # single whole-slab DMA (short startup critical path)
            for t in torder:
                ps = ppool.tile([128, 80], F32, tag="pnode")
                nc.tensor.matmul(ps[:], zT[0:16, t * 128:(t + 1) * 128],
                                 w_sb[0], start=True, stop=True)
                nc.scalar.copy(ntab_sb[:, t, 0:80], ps[:])
            nc.scalar.activation(cvals[:, :, 0:8], ntab_sb[:, :, 72:80],
                                 AF.Exp, scale=-0.8)
            nc.scalar.activation(ntab_sb[:, :, 72:80], ntab_sb[:, :, 64:72],
                                 AF.Exp, scale=0.2)
            nc.scalar.activation(ntab_sb[:, :, 64:72], ntab_sb[:, :, 64:72],
                                 AF.Exp)
            nc.sync.dma_start(
                slab123[0:NPAD, :].rearrange("(t p) f -> p t f", p=128),
                ntab_sb[:, :, :])
            finish_tables(0)

            ps16 = phpool.tile([16, 64], F32, tag="pshead")

            for l in range(4):
                heads, ch, cin, cout = LCFG[l]
                mw = heads * ch              # zbuf width (64 / 256)
                s2 = S_CHUNK[l]
                tab = tabs[l]

                for t in torder:
                    # ---- self-loop term initialises zbuf/denom ----
                    nc.vector.tensor_tensor(
                        msel[:, t, 0:heads],
                        ntab_sb[:, t, 64 + heads:64 + 2 * heads],
                        cvals[:, t, 0:heads], ALU.mult)
                    nc.vector.tensor_tensor(
                        msel[:, t, 0:heads],
                        ntab_sb[:, t, 64:64 + heads],
                        msel[:, t, 0:heads], ALU.max)
                    nc.scalar.copy(denom[:, t, 0:heads],
                                   msel[:, t, 0:heads])
                    if l < 3:
                        sf = ntab_sb[:, t, 0:64].rearrange(
                            "p (h c) -> p h c", h=heads)
                    else:
                        sf = ntab_sb[:, t, 0:64].unsqueeze(1).broadcast_to(
                            [128, heads, ch])
                    nc.vector.tensor_tensor(
                        zbuf[:, t, 0:mw].rearrange("p (h c) -> p h c",
                                                   h=heads),
                        sf,
                        msel[:, t, 0:heads].unsqueeze(2).broadcast_to(
                            [128, heads, ch]), ALU.mult)

                    # ---- edge chunks ----
                    for s0 in range(0, S[t], s2):
                        sn = min(s2, S[t] - s0)
                        g = gpool.tile([128, 32, 128], BF16, tag="g")
                        icols = idxe_sb[:, col0[t] + 8 * s0:
                                        col0[t] + 8 * (s0 + sn)]
                        nc.gpsimd.dma_gather(
                            g[:, 0:sn, :], tab[:, :], icols,
                            sn * 128, sn * 128, 128, single_packet=False,
                            queue_num=gq[0] % 4)
                        gq[0] += 1
                        mt = tpool.tile([128, s2, 8], BF16, tag="mt")
                        cb = cvals[:, t, 0:heads].unsqueeze(1)
                        cb = cb.broadcast_to([128, sn, heads])
                        nc.vector.tensor_tensor(
                            mt[:, 0:sn, 0:heads],
                            g[:, 0:sn, 64 + heads:64 + 2 * heads],
                            cb, ALU.mult)
                        nc.vector.tensor_tensor(
                            mt[:, 0:sn, 0:heads],
                            g[:, 0:sn, 64:64 + heads],
                            mt[:, 0:sn, 0:heads], ALU.max)
                        dt = tpool.tile([128, 8], F32, tag="dt")
                        nc.vector.tensor_reduce(
                            dt[:, 0:heads],
                            mt[:, 0:sn, 0:heads].transpose([0, 2, 1]),
                            mybir.AxisListType.X, ALU.add)
                        nc.vector.tensor_tensor(
                            denom[:, t, 0:heads], denom[:, t, 0:heads],
                            dt[:, 0:heads], ALU.add)
                        mg = mpool.tile([128, s2, mw], BF16, tag="mg")
                        if l < 3:
                            gf = g[:, 0:sn, 0:64].rearrange(
                                "p s (h c) -> p s h c", h=heads)
                        else:
                            gf = g[:, 0:sn, 0:64].unsqueeze(2).broadcast_to(
                                [128, sn, heads, ch])
                        nc.vector.tensor_tensor(
                            mg[:, 0:sn, :].rearrange(
                                "p s (h c) -> p s h c", h=heads),
                            gf,
                            mt[:, 0:sn, 0:heads].unsqueeze(3).broadcast_to(
                                [128, sn, heads, ch]), ALU.mult)
                        # slot reduction: contiguous halving tree (fast 16-bit
                        # DVE path) instead of a strided transpose-reduce
                        w = sn
                        while w > 1:
                            h2 = w // 2
                            if w % 2:
                                nc.vector.tensor_tensor(
                                    mg[:, 0, :], mg[:, 0, :],
                                    mg[:, w - 1, :], ALU.add)
                            nc.vector.tensor_tensor(
                                mg[:, 0:h2, :], mg[:, 0:h2, :],
                                mg[:, h2:2 * h2, :], ALU.add)
                            w = h2
                        nc.vector.tensor_tensor(
                            zbuf[:, t, 0:mw], zbuf[:, t, 0:mw],
                            mg[:, 0, :], ALU.add)

                    # ---- per-tile epilogue ----
                    # (denom >= A_self = exp(asrc) > 0: no clamp needed)
                    nc.vector.reciprocal(drec[:, t, 0:heads],
                                         denom[:, t, 0:heads])
                    zt4 = zbuf[:, t, 0:mw].rearrange("p (h c) -> p h c",
                                                     h=heads)
                    nc.vector.tensor_tensor(
                        zt4, zt4,
                        drec[:, t, 0:heads].unsqueeze(2).broadcast_to(
                            [128, heads, ch]), ALU.mult)
                    if l < 3:
                        nc.scalar.copy(zm[:, t, :], zbuf[:, t, 0:64])
                    else:
                        # z4pre = u_cat @ Wst (K=256 via 2 chunks)
                        pz = papool.tile([128, 64], F32, tag="pz4")
                        for j in range(2):
                            ut = ptpool.tile([128, 128], F32, tag="pt")
                            nc.tensor.transpose(
                                ut[:], zbuf[:, t, j * 128:(j + 1) * 128],
                                ident_sb[:])
                            us = tpool.tile([128, 128], F32, tag="us")
                            nc.scalar.copy(us[:], ut[:])
                            nc.tensor.matmul(pz[:], us[:], wst_sb(j),
                                             start=(j == 0), stop=(j == 1))
                        nc.scalar.copy(zm[:, t, :], pz[:])
                    # bias + elu(x) = relu(x) + exp(-relu(-x)) - 1
                    e1 = tpool.tile([128, 64], F32, tag="e1")
                    e2 = tpool.tile([128, 64], F32, tag="e2")
                    nc.vector.tensor_tensor(zm[:, t, :], zm[:, t, :],
                                            b_sb[l], ALU.add)
                    nc.scalar.activation(e1[:], zm[:, t, :], AF.Relu)
                    nc.scalar.activation(e2[:], zm[:, t, :], AF.Relu,
                                         scale=-1.0)
                    nc.scalar.activation(e2[:], e2[:], AF.Exp, scale=-1.0)
                    nc.vector.tensor_tensor(zm[:, t, :], e1[:], e2[:],
                                            ALU.add)
                    nc.scalar.activation(zm[:, t, :], zm[:, t, :], AF.Identity,
                                         bias=negone[:, 0:1])
                    if t == NTILE - 1 and NOWN % 128:
                        nc.scalar.mul(zm[:, t, :], zm[:, t, :],
                                      padmask_sb)
                    if l < 3:
                        pt = ptpool.tile([64, 128], F32, tag="pt")
                        nc.tensor.transpose(pt[:], zm[:, t, :], ident_sb[:])
                        nc.scalar.copy(zT[0:64, t * 128:(t + 1) * 128],
                                       pt[:])
                        node_tile(l + 1, t)
                    else:
                        # head pooling: [fp | tg] one-hot matmul, PSUM acc
                        nc.tensor.matmul(ps16[:], fdv_sb(t), zm[:, t, :],
                                         start=(t == torder[0]),
                                         stop=(t == torder[-1]))

                if l < 3:
                    finish_tables(l + 1)

            # ---- head: tiny AllReduce of [fp(8) | tg(8)] x 64 ----
            harr = tpool.tile([16, 64], F32, tag="harr")
            nc.scalar.copy(harr[:], ps16[:])
            arin = dram.tile([16, 64], F32, tag="arin")
            arout = dram.tile([16, 64], F32, tag="arout",
                              addr_space="Shared")
            nc.sync.dma_start(arin[:, :], harr[:])
            nc.gpsimd.collective_compute(
                "AllReduce", ALU.add,
                replica_groups=[list(range(NCORES))],
                ins=[arin[:, :].opt()],
                outs=[arout[:, :].opt()],
            )
            hb2 = tpool.tile([16, 64], F32, tag="hb2")
            nc.sync.dma_start(hb2[:], arout[:, :])

            # transpose [16, 64] -> [64, 16]: cols 0:8 fp, 8:16 tg
            pt16 = ptpool.tile([64, 16], F32, tag="pt")
            nc.tensor.transpose(pt16[:], hb2[:], ident16)
            ht = tpool.tile([64, 16], F32, tag="ht")
            nc.scalar.copy(ht[:], pt16[:])

            fp2 = ppool.tile([8, 64], F32, tag="h2nd")
            nc.tensor.matmul(fp2[:], ht[:, 0:8], wp_sb, start=True, stop=True)
            tg2 = ppool.tile([8, 64], F32, tag="h2nd")
            nc.tensor.matmul(tg2[:], ht[:, 8:16], wt_sb, start=True, stop=True)

            def transpose_8x64(src_psum, tag):
                s8 = tpool.tile([8, 64], F32, tag=tag + "s")
                nc.scalar.copy(s8[:], src_psum)
                pt = ptpool.tile([64, 8], F32, tag="pt")
                nc.tensor.transpose(pt[:], s8[:], ident8)
                o = tpool.tile([64, 8], F32, tag=tag + "o")
                nc.scalar.copy(o[:], pt[:])
                return o

            zh = tpool.tile([128, 8], F32, tag="zh")
            f2T = transpose_8x64(fp2[:], "f2")
            t2T = transpose_8x64(tg2[:], "t2")
            nc.scalar.copy(zh[0:64, :], f2T[:])
            nc.scalar.copy(zh[64:128, :], t2T[:])
            h1 = tpool.tile([128, 8], F32, tag="h1")
            h2 = tpool.tile([128, 8], F32, tag="h2")
            nc.scalar.activation(h1[:], zh[:], AF.Relu)
            nc.scalar.activation(h2[:], zh[:], AF.Relu, scale=-1.0)
            nc.scalar.activation(h2[:], h2[:], AF.Exp, scale=-1.0)
            nc.vector.tensor_tensor(zh[:], h1[:], h2[:], ALU.add)
            nc.scalar.activation(zh[:], zh[:], AF.Identity,
                                 bias=negone[:, 0:1])

            fin = ppool.tile([8, 1], F32, tag="h2nd")
            nc.tensor.matmul(fin[:], zh[:], wo_sb, start=True, stop=True)
            fo = tpool.tile([8, 1], F32, tag="fo")
            nc.scalar.activation(fo[:], fin[:], AF.Identity, bias=bo8)
            nc.sync.dma_start(out_final[:, :], fo[:])

    nc.compile()
    return nc


def prepare(x, edge_index, function_idx, flag, decision_var_idxes,
            W1, a_src1, a_dst1, b1, W2, a_src2, a_dst2, b2,
            W3, a_src3, a_dst3, b3, W4, a_src4, a_dst4, b4,
            Wp, Wt, Wo, bo):
    """Host preprocessing + program build -> (nc, in_maps)."""
    x = np.asarray(x, np.float32)
    pp = _preprocess(edge_index, function_idx, flag, decision_var_idxes)
    S, idx_tabs = pp["S"], pp["idx_tabs"]
    idx_width = idx_tabs[0].shape[1]

    nc = _build_kernel(S, idx_width)

    # host-side packed constant block
    wa = [(W1, a_src1, a_dst1), (W2, a_src2, a_dst2),
          (W3, a_src3, a_dst3)]
    cpack = np.zeros((128, CW), np.float32)
    for l, (W, asr, ads) in enumerate(wa):
        W = np.asarray(W, np.float32)
        asr = np.asarray(asr, np.float32)
        ads = np.asarray(ads, np.float32)
        wext = np.concatenate(
            [W, W @ _block_diag(asr), W @ _block_diag(ads)], axis=1)
        cpack[0:wext.shape[0], OFF_WEXT[l]:OFF_WEXT[l] + 80] = wext
    W4 = np.asarray(W4, np.float32)
    wsd4 = np.concatenate(
        [W4 @ _block_diag(np.asarray(a_src4, np.float32)),
         W4 @ _block_diag(np.asarray(a_dst4, np.float32))],
        axis=1).astype(np.float32)                     # [64, 8]
    cpack[0:64, OFF_WSD4:OFF_WSD4 + 8] = wsd4
    wst = (W4.reshape(64, 4, 64).transpose(1, 0, 2).reshape(256, 64)
           / 4.0)                                      # [256, 64], mean folded
    wst = wst.reshape(2, 128, 64).transpose(1, 0, 2)   # [128, 2, 64]
    cpack[:, OFF_WST:OFF_WST + 128] = wst.reshape(128, 128)
    for l, b in enumerate((b1, b2, b3, b4)):
        cpack[:, OFF_B + 64 * l:OFF_B + 64 * (l + 1)] = np.tile(
            np.asarray(b, np.float32)[None, :], (128, 1))
    cpack[0:64, OFF_WP:OFF_WP + 64] = np.asarray(Wp, np.float32)
    cpack[0:64, OFF_WT:OFF_WT + 64] = np.asarray(Wt, np.float32)
    cpack[:, OFF_WO] = np.asarray(Wo, np.float32).reshape(-1)
    cpack[:, OFF_BO] = np.float32(np.asarray(bo).reshape(-1)[0])
    cpack[:, OFF_PADM] = (np.arange(128) < (NOWN % 128 or 128)
                          ).astype(np.float32)
    cpack[:, OFF_IDENT:OFF_IDENT + 128] = np.eye(128, dtype=np.float32)

    new_of_orig = pp["new_of_orig"]
    orig_of_new = np.empty(N_NODES, np.int64)
    orig_of_new[new_of_orig] = np.arange(N_NODES)

    in_maps = []
    for c in range(NCORES):
        xT = np.zeros((16, NPAD), np.float32)
        xo = x[orig_of_new[c * NOWN:(c + 1) * NOWN]]  # [2500,16] local order
        xT[:, 0:NOWN] = xo.T
        cp = cpack.copy()
        cp[:, OFF_FDV:OFF_FDV + NTILE * 16] = \
            pp["fdv"][c].transpose(0, 1, 2).reshape(128, NTILE * 16)
        m = {"xT": xT, "idx_edges": idx_tabs[c], "cpack": cp}
        in_maps.append(m)
    return nc, in_maps


def kernel(**inputs):
    nc, in_maps = prepare(**inputs)
    trace = os.environ.get("GAT_TRACE", "0") == "1"
    res = run_bass_kernel_spmd(nc, in_maps, core_ids=list(range(NCORES)),
                               trace=trace)
    global last_results
    last_results = res
    out = res.results[0]["out_final"].astype(np.float32)
    return out


last_results = None


# revision 24
# speedup vs baseline: 1.1028x; 1.1028x over previous
"""GATNet (4-layer GAT + pooling head) on 8 Trainium2 NeuronCores.

Strategy (per sharding hint): partition nodes across the 8 cores; each core
owns a contiguous block of 2500 (renumbered) nodes and all edges whose dst
lands in that block.  Per layer each core computes its own node table
(h@W plus attention-logit exponentials); the tables are AllGathered into a
full HBM gather table (in chunks, overlapped with the previous layer's edge
phase), and each core gathers the source-node rows for its incoming edges
with dma_gather (dst-lane layout: edge slot s of dst-lane p lands at
partition p, free block s).  Edge softmax is restructured so no per-edge
dst-indexed data is ever needed:

    ref:  e = leaky_relu(asrc[src]+adst[dst]); alpha = softmax_dst(e)
    here: m = max(exp(asrc[src]), exp(0.2*asrc[src]) * exp(-0.8*adst[dst]))
          == exp(leaky_relu(asrc+adst)) * exp(-adst)   (dst factor cancels
          in the softmax), with exp(asrc)/exp(0.2*asrc) carried inside the
          gathered row and exp(-0.8*adst) a per-own-node column broadcast.

Segment sums become per-partition free-dim reductions.  The pooling head
(function-node sum pool + decision-var select) is folded into per-tile
one-hot matmuls accumulated in PSUM during the layer-4 loop, followed by a
single tiny AllReduce.  All graph index tables are built on the host
(compile-time) from the runtime edge_index.
"""

import os
import sys

import numpy as np

for _p in ("/opt/trn_rl_repo", "/root/.axon_site/_ro/trn_rl_repo"):
    if os.path.isdir(_p) and _p not in sys.path:
        sys.path.insert(0, _p)

import concourse.bacc as bacc
import concourse.bass as bass
import concourse.tile as tile
from concourse import mybir
from concourse.bass_utils import run_bass_kernel_spmd

F32 = mybir.dt.float32
BF16 = mybir.dt.float16  # fp16: 11-bit mantissa, plenty of range here
I16 = mybir.dt.int16
AF = mybir.ActivationFunctionType
ALU = mybir.AluOpType

N_NODES = 20000
N_EDGES = 320000
NCORES = 8
NOWN = N_NODES // NCORES          # 2500
NTILE = (NOWN + 127) // 128       # 20 dst tiles per core
NPAD = NTILE * 128                # 2560 padded own nodes
SLABR = NPAD + 16                 # per-core slab rows (last 16 all-zero)
DUMMY_ROW = NPAD                  # core 0's first zero row
TAB_ROWS = NCORES * SLABR         # 20608

# layer configs: (heads, ch, cin, cout)
LCFG = [(8, 8, 16, 64), (8, 8, 64, 64), (8, 8, 64, 64), (4, 64, 64, 256)]

# edge-phase slot-chunk sizes (free-dim S per gather): keep SBUF bounded
S_CHUNK = [48, 48, 48, 24]

# packed f32 constant-block column offsets
OFF_WEXT = [0, 80, 160]           # [cin, 80] each
OFF_WSD4 = 240                    # [64, 8]
OFF_WST = 248                     # [128, 2*64]
OFF_B = 376                       # 4 x [128, 64]
OFF_WP = 632                      # [64, 64]
OFF_WT = 696                      # [64, 64]
OFF_WO = 760                      # [128, 1]
OFF_BO = 761                      # [128, 1]
OFF_PADM = 762                    # [128, 1]
OFF_IDENT = 763                   # [128, 128]
OFF_FDV = 891                     # [128, NTILE*16]
CW = OFF_FDV + NTILE * 16         # 1211


def _idx_layout(flat):
    """Gather idx order i -> SBUF int16 [128, len/16] (pos [i%16, i//16],
    replicated over the 8 q7 core groups)."""
    flat = np.asarray(flat, np.int16)
    assert len(flat) % 16 == 0
    a = flat.reshape(-1, 16).T  # [16, n/16]
    return np.tile(a, (8, 1)).copy()


def _block_diag(a):
    """a [heads, ch] -> [heads*ch, heads] with column h = a[h] on its block."""
    heads, ch = a.shape
    out = np.zeros((heads * ch, heads), np.float32)
    for h in range(heads):
        out[h * ch:(h + 1) * ch, h] = a[h]
    return out


def _rowid(g):
    """tab row of renumbered-global node id (t-major rows within core)."""
    c, l = g // NOWN, g % NOWN
    return c * SLABR + l  # l == t*128 + p, t-major



def _preprocess(edge_index, function_idx, flag, decision_var_idxes):
    # NOTE: the appended self-loops are NOT put in the edge tables —
    # the device adds the self term analytically from local node data.
    src = np.asarray(edge_index[0], np.int64)
    dst = np.asarray(edge_index[1], np.int64)

    deg = np.bincount(dst, minlength=N_NODES)

    # renumber: global degree-desc order, round-robin over cores, so every
    # core's tile t covers the same global degree band (minimises slot pad)
    ranks = np.argsort(-deg, kind="stable")
    new_of_orig = np.empty(N_NODES, np.int64)
    gi = np.arange(N_NODES)
    new_of_orig[ranks] = (gi % NCORES) * NOWN + gi // NCORES

    src_n = new_of_orig[src]
    dst_n = new_of_orig[dst]

    deg_n = np.zeros(N_NODES, np.int64)
    np.add.at(deg_n, dst_n, 1)

    # shared tile slot counts S_t = max over cores of max degree in tile
    degs_2d = deg_n.reshape(NCORES, NOWN)
    S = []
    for t in range(NTILE):
        hi = min((t + 1) * 128, NOWN)
        S.append(int(degs_2d[:, t * 128:hi].max()))
    S = [max(s, 1) for s in S]

    # per-core edge slot tables A[l, s] = rowid(src) (DUMMY_ROW pad)
    order = np.argsort(dst_n, kind="stable")
    dst_s, src_s = dst_n[order], src_n[order]
    starts = np.searchsorted(dst_s, np.arange(N_NODES))
    slot = np.arange(len(dst_s)) - starts[dst_s]
    src_row = _rowid(src_s)

    idx_tabs = []
    for c in range(NCORES):
        m = (dst_s >= c * NOWN) & (dst_s < (c + 1) * NOWN)
        l = dst_s[m] - c * NOWN
        A = np.full((NPAD, max(S)), DUMMY_ROW, np.int64)
        A[l, slot[m]] = src_row[m]
        parts = []
        for t in range(NTILE):
            blk = A[t * 128:(t + 1) * 128, :S[t]].T  # [S_t, 128]
            parts.append(blk.reshape(-1))
        idx_tabs.append(_idx_layout(np.concatenate(parts)))

    # head one-hots: fdv[c, p, t, 0:8] = function-pool multiplicity per
    # graph; fdv[c, p, t, 8+j] = 1 if node is decision var of graph j
    fdv = np.zeros((NCORES, 128, NTILE, 16), np.float32)
    fidx_new = new_of_orig[np.asarray(function_idx, np.int64)]
    fc, fl = fidx_new // NOWN, fidx_new % NOWN
    ft, fp = fl // 128, fl % 128
    np.add.at(fdv, (fc, fp, ft, np.asarray(flag, np.int64)), 1.0)
    dvn = new_of_orig[np.asarray(decision_var_idxes, np.int64)]
    dc, dl = dvn // NOWN, dvn % NOWN
    dt, dp = dl // 128, dl % 128
    for j in range(NCORES):
        fdv[dc[j], dp[j], dt[j], 8 + j] += 1.0

    return dict(new_of_orig=new_of_orig, S=S, idx_tabs=idx_tabs, fdv=fdv)


def _build_kernel(S, idx_width):
    """Build the SPMD bass program (same for all cores).

    Per-layer gather-table rows are uniformly 128 fp16 (=256B):
    [feat(64) | asrc(h) | adst(h) | 0-pad], where feat is hW for layers
    1-3 and raw h3 for layer 4 (W4 is applied after aggregation via
    linearity: sum_e m_e*(h3@W4) == (sum_e m_e*h3)@W4).
    """
    nc = bacc.Bacc("TRN2", target_bir_lowering=False, debug=False,
                   num_devices=NCORES, num_swdge_queues=4)

    # ---- external inputs ----
    xT = nc.dram_tensor("xT", [16, NPAD], F32, kind="ExternalInput")
    idx_edges = nc.dram_tensor("idx_edges", [128, idx_width], I16,
                               kind="ExternalInput")
    cpack_d = nc.dram_tensor("cpack", [128, CW], F32, kind="ExternalInput")

    out_final = nc.dram_tensor("out_final", [8, 1], F32, kind="ExternalOutput")

    with tile.TileContext(nc) as tc:
        torder = list(range(NTILE))
        gq = [0]  # rotating SWDGE queue: each queue uses its own Q7 core
        # pair, so gathers on different queues generate descriptors in
        # parallel (dma_gather ucode: cpu_id/2 == queue_num does the work)
        with (
            tc.tile_pool(name="dram", bufs=1, space="DRAM") as dram,
            tc.tile_pool(name="const", bufs=1) as cpool,
            tc.tile_pool(name="state", bufs=1) as spool,
            tc.tile_pool(name="gather", bufs=6) as gpool,
            tc.tile_pool(name="msg", bufs=4) as mpool,
            tc.tile_pool(name="small", bufs=6) as tpool,
            tc.tile_pool(name="psum", bufs=2, space="PSUM") as ppool,
            tc.tile_pool(name="psacc", bufs=1, space="PSUM") as papool,
            tc.tile_pool(name="psumT", bufs=2, space="PSUM") as ptpool,
            tc.tile_pool(name="pshead", bufs=1, space="PSUM") as phpool,
        ):
            # ---- DRAM internals ----
            slab123 = dram.tile([SLABR, 128], BF16, tag="slab123")
            slab4 = dram.tile([SLABR, 128], BF16, tag="slab4")
            tabs = []
            for _l in range(4):
                tab_l = dram.tile([TAB_ROWS, 128], BF16, tag=f"tab{_l}",
                                  addr_space="Shared", name=f"tab{_l}")
                tabs.append(tab_l)

            # ---- load constants (one packed DMA + the idx table) ----
            cpk = cpool.tile([128, CW], F32, tag="cpk")
            nc.sync.dma_start(cpk[:], cpack_d[:, :])
            idxe_sb = cpool.tile([128, idx_width], I16, tag="idxe")
            nc.sync.dma_start(idxe_sb[:], idx_edges[:, :])

            w_sb = [cpk[0:(64 if l else 16), OFF_WEXT[l]:OFF_WEXT[l] + 80]
                    for l in range(3)]
            wsd4_sb = cpk[0:64, OFF_WSD4:OFF_WSD4 + 8]
            b_sb = [cpk[:, OFF_B + 64 * l:OFF_B + 64 * (l + 1)]
                    for l in range(4)]
            wp_sb = cpk[0:64, OFF_WP:OFF_WP + 64]
            wt_sb = cpk[0:64, OFF_WT:OFF_WT + 64]
            wo_sb = cpk[:, OFF_WO:OFF_WO + 1]
            bo8 = cpk[0:8, OFF_BO:OFF_BO + 1]
            padmask_sb = cpk[:, OFF_PADM:OFF_PADM + 1]
            ident_sb = cpk[:, OFF_IDENT:OFF_IDENT + 128]
            ident16 = cpk[0:16, OFF_IDENT:OFF_IDENT + 16]
            ident8 = cpk[0:8, OFF_IDENT:OFF_IDENT + 8]

            def wst_sb(j):
                return cpk[:, OFF_WST + 64 * j:OFF_WST + 64 * (j + 1)]

            def fdv_sb(t):
                return cpk[:, OFF_FDV + 16 * t:OFF_FDV + 16 * (t + 1)]

            zero_sb = cpool.tile([128, 128], BF16, tag="zero")
            nc.vector.memset(zero_sb[:], 0.0)
            negone = cpool.tile([128, 1], F32, tag="negone")
            nc.vector.memset(negone[:], -1.0)
            # zero the trailing pad rows of each slab (gather dummy target)
            nc.sync.dma_start(slab123[NPAD:SLABR, :], zero_sb[0:16, :])
            nc.sync.dma_start(slab4[NPAD:SLABR, :], zero_sb[0:16, :])

            # ---- persistent state ----
            zT = spool.tile([64, NPAD], F32, tag="zT")
            nc.vector.memset(zT[:], 0.0)
            nc.sync.dma_start(zT[0:16, :], xT[:, :])

            ntab_sb = spool.tile([128, NTILE, 128], BF16, tag="ntab")
            nc.vector.memset(ntab_sb[:], 0.0)
            msel = spool.tile([128, NTILE, 8], F32, tag="msel")
            zbuf = spool.tile([128, NTILE, 256], F32, tag="zbuf")
            zm = spool.tile([128, NTILE, 64], F32, tag="zm")
            cvals = spool.tile([128, NTILE, 8], BF16, tag="cvals")
            denom = spool.tile([128, NTILE, 8], F32, tag="denom")
            drec = spool.tile([128, NTILE, 8], F32, tag="drec")

            col0 = np.cumsum([0] + [8 * s for s in S]).tolist()

            def node_tile(l, t):
                """ntab rows [feat|asrc|adst] + exps + slab write, tile t."""
                h = LCFG[l][0]
                if l < 3:
                    cin = 64 if l else 16
                    ps = ppool.tile([128, 80], F32, tag="pnode")
                    nc.tensor.matmul(ps[:], zT[0:cin, t * 128:(t + 1) * 128],
                                     w_sb[l], start=True, stop=True)
                    nc.scalar.copy(ntab_sb[:, t, 0:80], ps[:])
                else:
                    nc.scalar.copy(ntab_sb[:, t, 0:64], zm[:, t, :])
                    ps = ppool.tile([128, 8], F32, tag="pnode")
                    nc.tensor.matmul(ps[:], zT[0:64, t * 128:(t + 1) * 128],
                                     wsd4_sb, start=True, stop=True)
                    nc.scalar.copy(ntab_sb[:, t, 64:72], ps[:])
                asrc = ntab_sb[:, t, 64:64 + h]
                adst = ntab_sb[:, t, 64 + h:64 + 2 * h]
                nc.scalar.activation(cvals[:, t, 0:h], adst, AF.Exp,
                                     scale=-0.8)
                nc.scalar.activation(adst, asrc, AF.Exp, scale=0.2)
                nc.scalar.activation(asrc, asrc, AF.Exp)
                slab = slab123 if l < 3 else slab4
                nc.sync.dma_start(slab[t * 128:(t + 1) * 128, :],
                                  ntab_sb[:, t, :])

            def finish_tables(l):
                slab = slab123 if l < 3 else slab4
                nc.gpsimd.collective_compute(
                    "AllGather", ALU.bypass,
                    replica_groups=[list(range(NCORES))],
                    ins=[slab[:, :].opt()],
                    outs=[tabs[l][:, :].opt()],
                )

            for t in torder:
                node_tile(0, t)
            finish_tables(0)

            ps16 = phpool.tile([16, 64], F32, tag="pshead")

            for l in range(4):
                heads, ch, cin, cout = LCFG[l]
                mw = heads * ch              # zbuf width (64 / 256)
                s2 = S_CHUNK[l]
                tab = tabs[l]

                for t in torder:
                    # ---- self-loop term initialises zbuf/denom ----
                    nc.vector.tensor_tensor(
                        msel[:, t, 0:heads],
                        ntab_sb[:, t, 64 + heads:64 + 2 * heads],
                        cvals[:, t, 0:heads], ALU.mult)
                    nc.vector.tensor_tensor(
                        msel[:, t, 0:heads],
                        ntab_sb[:, t, 64:64 + heads],
                        msel[:, t, 0:heads], ALU.max)
                    nc.scalar.copy(denom[:, t, 0:heads],
                                   msel[:, t, 0:heads])
                    if l < 3:
                        sf = ntab_sb[:, t, 0:64].rearrange(
                            "p (h c) -> p h c", h=heads)
                    else:
                        sf = ntab_sb[:, t, 0:64].unsqueeze(1).broadcast_to(
                            [128, heads, ch])
                    nc.vector.tensor_tensor(
                        zbuf[:, t, 0:mw].rearrange("p (h c) -> p h c",
                                                   h=heads),
                        sf,
                        msel[:, t, 0:heads].unsqueeze(2).broadcast_to(
                            [128, heads, ch]), ALU.mult)

                    # ---- edge chunks ----
                    for s0 in range(0, S[t], s2):
                        sn = min(s2, S[t] - s0)
                        g = gpool.tile([128, 48, 128], BF16, tag="g")
                        icols = idxe_sb[:, col0[t] + 8 * s0:
                                        col0[t] + 8 * (s0 + sn)]
                        nc.gpsimd.dma_gather(
                            g[:, 0:sn, :], tab[:, :], icols,
                            sn * 128, sn * 128, 128, single_packet=False,
                            queue_num=gq[0] % 4)
                        gq[0] += 1
                        mt = tpool.tile([128, s2, 8], BF16, tag="mt")
                        cb = cvals[:, t, 0:heads].unsqueeze(1)
                        cb = cb.broadcast_to([128, sn, heads])
                        nc.vector.tensor_tensor(
                            mt[:, 0:sn, 0:heads],
                            g[:, 0:sn, 64 + heads:64 + 2 * heads],
                            cb, ALU.mult)
                        nc.vector.tensor_tensor(
                            mt[:, 0:sn, 0:heads],
                            g[:, 0:sn, 64:64 + heads],
                            mt[:, 0:sn, 0:heads], ALU.max)
                        dt = tpool.tile([128, 8], F32, tag="dt")
                        nc.vector.tensor_reduce(
                            dt[:, 0:heads],
                            mt[:, 0:sn, 0:heads].transpose([0, 2, 1]),
                            mybir.AxisListType.X, ALU.add)
                        nc.vector.tensor_tensor(
                            denom[:, t, 0:heads], denom[:, t, 0:heads],
                            dt[:, 0:heads], ALU.add)
                        mg = mpool.tile([128, s2, mw], BF16, tag="mg")
                        if l < 3:
                            gf = g[:, 0:sn, 0:64].rearrange(
                                "p s (h c) -> p s h c", h=heads)
                        else:
                            gf = g[:, 0:sn, 0:64].unsqueeze(2).broadcast_to(
                                [128, sn, heads, ch])
                        nc.vector.tensor_tensor(
                            mg[:, 0:sn, :].rearrange(
                                "p s (h c) -> p s h c", h=heads),
                            gf,
                            mt[:, 0:sn, 0:heads].unsqueeze(3).broadcast_to(
                                [128, sn, heads, ch]), ALU.mult)
                        # slot reduction: contiguous halving tree (fast 16-bit
                        # DVE path) instead of a strided transpose-reduce
                        w = sn
                        while w > 1:
                            h2 = w // 2
                            if w % 2:
                                nc.vector.tensor_tensor(
                                    mg[:, 0, :], mg[:, 0, :],
                                    mg[:, w - 1, :], ALU.add)
                            nc.vector.tensor_tensor(
                                mg[:, 0:h2, :], mg[:, 0:h2, :],
                                mg[:, h2:2 * h2, :], ALU.add)
                            w = h2
                        nc.vector.tensor_tensor(
                            zbuf[:, t, 0:mw], zbuf[:, t, 0:mw],
                            mg[:, 0, :], ALU.add)

                    # ---- per-tile epilogue ----
                    # (denom >= A_self = exp(asrc) > 0: no clamp needed)
                    nc.vector.reciprocal(drec[:, t, 0:heads],
                                         denom[:, t, 0:heads])
                    zt4 = zbuf[:, t, 0:mw].rearrange("p (h c) -> p h c",
                                                     h=heads)
                    nc.vector.tensor_tensor(
                        zt4, zt4,
                        drec[:, t, 0:heads].unsqueeze(2).broadcast_to(
                            [128, heads, ch]), ALU.mult)
                    if l < 3:
                        nc.scalar.copy(zm[:, t, :], zbuf[:, t, 0:64])
                    else:
                        # z4pre = u_cat @ Wst (K=256 via 2 chunks)
                        pz = papool.tile([128, 64], F32, tag="pz4")
                        for j in range(2):
                            ut = ptpool.tile([128, 128], F32, tag="pt")
                            nc.tensor.transpose(
                                ut[:], zbuf[:, t, j * 128:(j + 1) * 128],
                                ident_sb[:])
                            us = tpool.tile([128, 128], F32, tag="us")
                            nc.scalar.copy(us[:], ut[:])
                            nc.tensor.matmul(pz[:], us[:], wst_sb(j),
                                             start=(j == 0), stop=(j == 1))
                        nc.scalar.copy(zm[:, t, :], pz[:])
                    # bias + elu(x) = relu(x) + exp(-relu(-x)) - 1
                    e1 = tpool.tile([128, 64], F32, tag="e1")
                    e2 = tpool.tile([128, 64], F32, tag="e2")
                    nc.vector.tensor_tensor(zm[:, t, :], zm[:, t, :],
                                            b_sb[l], ALU.add)
                    nc.scalar.activation(e1[:], zm[:, t, :], AF.Relu)
                    nc.scalar.activation(e2[:], zm[:, t, :], AF.Relu,
                                         scale=-1.0)
                    nc.scalar.activation(e2[:], e2[:], AF.Exp, scale=-1.0)
                    nc.vector.tensor_tensor(zm[:, t, :], e1[:], e2[:],
                                            ALU.add)
                    nc.scalar.activation(zm[:, t, :], zm[:, t, :], AF.Identity,
                                         bias=negone[:, 0:1])
                    if t == NTILE - 1 and NOWN % 128:
                        nc.scalar.mul(zm[:, t, :], zm[:, t, :],
                                      padmask_sb)
                    if l < 3:
                        pt = ptpool.tile([64, 128], F32, tag="pt")
                        nc.tensor.transpose(pt[:], zm[:, t, :], ident_sb[:])
                        nc.scalar.copy(zT[0:64, t * 128:(t + 1) * 128],
                                       pt[:])
                        node_tile(l + 1, t)
                    else:
                        # head pooling: [fp | tg] one-hot matmul, PSUM acc
                        nc.tensor.matmul(ps16[:], fdv_sb(t), zm[:, t, :],
                                         start=(t == torder[0]),
                                         stop=(t == torder[-1]))

                if l < 3:
                    finish_tables(l + 1)

            # ---- head: tiny AllReduce of [fp(8) | tg(8)] x 64 ----
            harr = tpool.tile([16, 64], F32, tag="harr")
            nc.scalar.copy(harr[:], ps16[:])
            arin = dram.tile([16, 64], F32, tag="arin")
            arout = dram.tile([16, 64], F32, tag="arout",
                              addr_space="Shared")
            nc.sync.dma_start(arin[:, :], harr[:])
            nc.gpsimd.collective_compute(
                "AllReduce", ALU.add,
                replica_groups=[list(range(NCORES))],
                ins=[arin[:, :].opt()],
                outs=[arout[:, :].opt()],
            )
            hb2 = tpool.tile([16, 64], F32, tag="hb2")
            nc.sync.dma_start(hb2[:], arout[:, :])

            # transpose [16, 64] -> [64, 16]: cols 0:8 fp, 8:16 tg
            pt16 = ptpool.tile([64, 16], F32, tag="pt")
            nc.tensor.transpose(pt16[:], hb2[:], ident16)
            ht = tpool.tile([64, 16], F32, tag="ht")
            nc.scalar.copy(ht[:], pt16[:])

            fp2 = ppool.tile([8, 64], F32, tag="h2nd")
            nc.tensor.matmul(fp2[:], ht[:, 0:8], wp_sb, start=True, stop=True)
            tg2 = ppool.tile([8, 64], F32, tag="h2nd")
            nc.tensor.matmul(tg2[:], ht[:, 8:16], wt_sb, start=True, stop=True)

            def transpose_8x64(src_psum, tag):
                s8 = tpool.tile([8, 64], F32, tag=tag + "s")
                nc.scalar.copy(s8[:], src_psum)
                pt = ptpool.tile([64, 8], F32, tag="pt")
                nc.tensor.transpose(pt[:], s8[:], ident8)
                o = tpool.tile([64, 8], F32, tag=tag + "o")
                nc.scalar.copy(o[:], pt[:])
                return o

            zh = tpool.tile([128, 8], F32, tag="zh")
            f2T = transpose_8x64(fp2[:], "f2")
            t2T = transpose_8x64(tg2[:], "t2")
            nc.scalar.copy(zh[0:64, :], f2T[:])
            nc.scalar.copy(zh[64:128, :], t2T[:])
            h1 = tpool.tile([128, 8], F32, tag="h1")
            h2 = tpool.tile([128, 8], F32, tag="h2")
            nc.scalar.activation(h1[:], zh[:], AF.Relu)
            nc.scalar.activation(h2[:], zh[:], AF.Relu, scale=-1.0)
            nc.scalar.activation(h2[:], h2[:], AF.Exp, scale=-1.0)
            nc.vector.tensor_tensor(zh[:], h1[:], h2[:], ALU.add)
            nc.scalar.activation(zh[:], zh[:], AF.Identity,
                                 bias=negone[:, 0:1])

            fin = ppool.tile([8, 1], F32, tag="h2nd")
            nc.tensor.matmul(fin[:], zh[:], wo_sb, start=True, stop=True)
            fo = tpool.tile([8, 1], F32, tag="fo")
            nc.scalar.activation(fo[:], fin[:], AF.Identity, bias=bo8)
            nc.sync.dma_start(out_final[:, :], fo[:])

    nc.compile()
    return nc


def prepare(x, edge_index, function_idx, flag, decision_var_idxes,
            W1, a_src1, a_dst1, b1, W2, a_src2, a_dst2, b2,
            W3, a_src3, a_dst3, b3, W4, a_src4, a_dst4, b4,
            Wp, Wt, Wo, bo):
    """Host preprocessing + program build -> (nc, in_maps)."""
    x = np.asarray(x, np.float32)
    pp = _preprocess(edge_index, function_idx, flag, decision_var_idxes)
    S, idx_tabs = pp["S"], pp["idx_tabs"]
    idx_width = idx_tabs[0].shape[1]

    nc = _build_kernel(S, idx_width)

    # host-side packed constant block
    wa = [(W1, a_src1, a_dst1), (W2, a_src2, a_dst2),
          (W3, a_src3, a_dst3)]
    cpack = np.zeros((128, CW), np.float32)
    for l, (W, asr, ads) in enumerate(wa):
        W = np.asarray(W, np.float32)
        asr = np.asarray(asr, np.float32)
        ads = np.asarray(ads, np.float32)
        wext = np.concatenate(
            [W, W @ _block_diag(asr), W @ _block_diag(ads)], axis=1)
        cpack[0:wext.shape[0], OFF_WEXT[l]:OFF_WEXT[l] + 80] = wext
    W4 = np.asarray(W4, np.float32)
    wsd4 = np.concatenate(
        [W4 @ _block_diag(np.asarray(a_src4, np.float32)),
         W4 @ _block_diag(np.asarray(a_dst4, np.float32))],
        axis=1).astype(np.float32)                     # [64, 8]
    cpack[0:64, OFF_WSD4:OFF_WSD4 + 8] = wsd4
    wst = (W4.reshape(64, 4, 64).transpose(1, 0, 2).reshape(256, 64)
           / 4.0)                                      # [256, 64], mean folded
    wst = wst.reshape(2, 128, 64).transpose(1, 0, 2)   # [128, 2, 64]
    cpack[:, OFF_WST:OFF_WST + 128] = wst.reshape(128, 128)
    for l, b in enumerate((b1, b2, b3, b4)):
        cpack[:, OFF_B + 64 * l:OFF_B + 64 * (l + 1)] = np.tile(
            np.asarray(b, np.float32)[None, :], (128, 1))
    cpack[0:64, OFF_WP:OFF_WP + 64] = np.asarray(Wp, np.float32)
    cpack[0:64, OFF_WT:OFF_WT + 64] = np.asarray(Wt, np.float32)
    cpack[:, OFF_WO] = np.asarray(Wo, np.float32).reshape(-1)
    cpack[:, OFF_BO] = np.float32(np.asarray(bo).reshape(-1)[0])
    cpack[:, OFF_PADM] = (np.arange(128) < (NOWN % 128 or 128)
                          ).astype(np.float32)
    cpack[:, OFF_IDENT:OFF_IDENT + 128] = np.eye(128, dtype=np.float32)

    new_of_orig = pp["new_of_orig"]
    orig_of_new = np.empty(N_NODES, np.int64)
    orig_of_new[new_of_orig] = np.arange(N_NODES)

    in_maps = []
    for c in range(NCORES):
        xT = np.zeros((16, NPAD), np.float32)
        xo = x[orig_of_new[c * NOWN:(c + 1) * NOWN]]  # [2500,16] local order
        xT[:, 0:NOWN] = xo.T
        cp = cpack.copy()
        cp[:, OFF_FDV:OFF_FDV + NTILE * 16] = \
            pp["fdv"][c].transpose(0, 1, 2).reshape(128, NTILE * 16)
        m = {"xT": xT, "idx_edges": idx_tabs[c], "cpack": cp}
        in_maps.append(m)
    return nc, in_maps


def kernel(**inputs):
    nc, in_maps = prepare(**inputs)
    trace = os.environ.get("GAT_TRACE", "0") == "1"
    res = run_bass_kernel_spmd(nc, in_maps, core_ids=list(range(NCORES)),
                               trace=trace)
    global last_results
    last_results = res
    out = res.results[0]["out_final"].astype(np.float32)
    return out


last_results = None
